# revision 21
# baseline (speedup 1.0000x reference)
"""Fused conv1x1-attention-FFN kernel for 8 trn2 NeuronCores.

Reference computation (per batch b of 4, N = 64*64 = 4096 pixels, C = 256):
    q = Wq @ x_q + bq ; k = Wk @ x_kv + bk ; v = Wv @ x_kv + bv      [C, N]
    attn = softmax_over_keys(q^T k)                                   [N, N]
    av = v @ attn^T                                                   [C, N]
    out = W2 @ relu(W1 @ av + b1) + b2                                [C, N]

Sharding: 8 cores = 4 batches x 2 query-row halves. Each core computes the
full K/V for its batch (cheap vs the [N,N] attention) and attends its 2048
query rows against all 4096 keys — no collectives.

Bias algebra (host-side): bk shifts every score of a query equally, which
softmax cancels -> dropped. bv adds bv per channel to the normalized
attention output (weights sum to 1) -> folded into b1' = b1 + W1 @ bv.

On-chip layout: scores are computed TRANSPOSED, S^T[m,n] = sum_c k[c,m]
q[c,n], so av[c,n] = sum_m v^T[m,c] E[m,n] needs no transpose; v is
projected into v^T[m,c] by using x_kv tiles as the stationary operand.
Softmax denominators are accumulated on the Vector engine (acc += exp
tile) and reduced over keys with ONE 1-column matmul per chunk.

Weight algebra (host-side): the k-projection is eliminated entirely —
scores are bilinear, so S^T = x_kv^T (Wqk x_q + Wk^T bq) with Wqk =
Wk^T Wq precomputed on host; the score matmuls use raw x_kv tiles as the
stationary operand. Weights are packed (Wqk,Wv,W1,W2) and DMA'd in
slices interleaved with the input chunks so the q-projection starts as
soon as Wqk plus 512 columns of x_q have landed; chunk-0 attention
streams right behind the projections, with the av/acc consumption skewed
SKEW key-tiles behind the score/exp pipeline so the av matmuls never
wait on ACT latency. Junk warm-up matmuls run during the initial DMA
window (and through the latency-bound tail) to keep the PE HAM clock
gate released at 2.4 GHz. Tail: the last chunk's FFN runs UN-normalized — relu(s*z
+ b1) = s*relu(z + b1*denom) for s>0 — with b1*denom / b2*denom added via
rank-1 matmuls into PSUM, so the reciprocal/broadcast chain overlaps the
FFN matmuls instead of preceding them; a final elementwise multiply by
the broadcast reciprocal lands the result.

Everything DMAs straight into float32r tiles (bit-identical to f32) — no
staging casts. float32r matmuls run 1 PE cycle/row at free-dim >= 256;
PSUM accumulates fp32.
"""
import sys

sys.path.insert(0, "/opt/trn_rl_repo")

import numpy as np
from concourse import bass, bacc, mybir, tile
from concourse.bass_utils import run_bass_kernel_spmd

F32 = mybir.dt.float32
CDT = mybir.dt.float32r

B, C, H, W = 4, 256, 64, 64
N = H * W              # 4096 keys per batch
NL = N // 2            # 2048 query rows per core
CT = C // 128          # 2 channel tiles
MT = N // 128          # 32 key tiles
NCH = 512              # query-column chunk
NJ = NL // NCH         # 4 chunks
AF = mybir.ActivationFunctionType
BQ, B1, B2 = range(3)  # bias pack columns
WQK, WV, W1, W2 = range(4)  # weight pack order (DMA priority order)


def _build():
    nc = bacc.Bacc(None, target_bir_lowering=False, debug=False)

    xq_d = nc.declare_dram_parameter("xq", [128, CT, NL], CDT, isOutput=False)
    xkv_d = nc.declare_dram_parameter("xkv", [128, CT, N], CDT, isOutput=False)
    wp_d = nc.declare_dram_parameter("wpack", [128, CT, 4 * C], CDT, isOutput=False)
    bias_d = nc.declare_dram_parameter("biasp", [128, CT, 3], F32, isOutput=False)
    brow_d = nc.declare_dram_parameter("biasrow", [1, 2 * C], CDT, isOutput=False)
    out_d = nc.declare_dram_parameter("out", [128, CT, NL], F32, isOutput=True)

    with tile.TileContext(nc) as tc:
        with (
            tc.tile_pool(name="const", bufs=1) as cpool,
            tc.tile_pool(name="big", bufs=1) as bpool,
            tc.tile_pool(name="work", bufs=2) as wpool,
            tc.tile_pool(name="et", bufs=5) as epool,
            tc.tile_pool(name="acc", bufs=2) as apool,
            tc.tile_pool(name="psA", bufs=2, space="PSUM") as pp,
            tc.tile_pool(name="psB", bufs=3, space="PSUM") as pp2,
        ):
            w_r = cpool.tile([128, CT, 4 * C], CDT, tag="w_r")
            bias_s = cpool.tile([128, CT, 3], F32, tag="bias_s")
            brow = cpool.tile([1, 2 * C], CDT, tag="brow")
            xkv_r = bpool.tile([128, CT, N], CDT, tag="xkv_r")
            xq_r = bpool.tile([128, CT, NL], CDT, tag="xq_r")

            def wslice(i0, i1):
                return (w_r[:, :, i0 * C:i1 * C], wp_d[:, :, i0 * C:i1 * C])

            # DMA issue order == arrival order; interleave so the first
            # projections can start after ~1.3MB instead of ~2.6MB.
            nc.sync.dma_start(*wslice(WQK, WQK + 1))
            nc.sync.dma_start(xq_r[:, :, 0:512], xq_d[:, :, 0:512])
            nc.sync.dma_start(*wslice(WV, WV + 1))
            nc.sync.dma_start(xkv_r[:, :, 0:512], xkv_d[:, :, 0:512])
            nc.sync.dma_start(bias_s[:], bias_d[:])
            nc.sync.dma_start(xkv_r[:, :, 512:1024], xkv_d[:, :, 512:1024])
            nc.sync.dma_start(*wslice(W1, W2 + 1))
            nc.sync.dma_start(xq_r[:, :, 512:1024], xq_d[:, :, 512:1024])
            for c in range(1, 4):
                nc.sync.dma_start(xkv_r[:, :, c * 1024:(c + 1) * 1024],
                                  xkv_d[:, :, c * 1024:(c + 1) * 1024])
                if c == 1:
                    nc.sync.dma_start(xq_r[:, :, 1024:2048],
                                      xq_d[:, :, 1024:2048])
            nc.sync.dma_start(brow[:], brow_d[:])  # tail-only bias rows

            def wsl(idx, ct, osl):
                return w_r[:, ct, idx * C + osl.start: idx * C + osl.stop]

            # PE warm-up: the HAM clock gate starts at 1.2 GHz and needs
            # ~3.4us of sustained activity to release to 2.4 GHz. The PE is
            # otherwise idle 8-14us waiting for the first DMAs, so burn that
            # window on junk matmuls and start the real stream at full clock.
            junk_f = cpool.tile([128, NCH], F32, tag="junk_f")
            nc.vector.memset(junk_f[:], 1.0)
            junk = cpool.tile([128, NCH], CDT, tag="junk")
            nc.vector.tensor_copy(junk[:], junk_f[:])
            for wi in range(12):
                wp_ps = pp2.tile([128, NCH], F32, tag="st", name=f"warm{wi}")
                nc.tensor.matmul(wp_ps[:], junk[:, 0:128], junk[:],
                                 start=True, stop=True)

            ones_f = cpool.tile([128, 1], F32, tag="ones_f")
            nc.vector.memset(ones_f[:], 1.0)
            ones_r = cpool.tile([128, 1], CDT, tag="ones_r")
            nc.vector.tensor_copy(ones_r[:], ones_f[:])
            onesrow_f = cpool.tile([1, 128], F32, tag="onesrow_f")
            nc.vector.memset(onesrow_f[:], 1.0)
            onesrow = cpool.tile([1, 128], CDT, tag="onesrow")
            nc.vector.tensor_copy(onesrow[:], onesrow_f[:])

            q_r = bpool.tile([128, CT, NL], CDT, tag="q_r")
            vt_r = bpool.tile([128, MT, C], CDT, tag="vt_r")

            def vtproj(mi):  # v^T tile: [key, channel] layout, no bias
                ps = pp2.tile([128, C], F32, tag="st", name=f"pv{mi}")
                for ci in range(CT):
                    nc.tensor.matmul(ps[:], xkv_r[:, ci, mi * 128:(mi + 1) * 128],
                                     w_r[:, ci, WV * C:WV * C + C], start=(ci == 0), stop=(ci == CT - 1))
                nc.scalar.activation(vt_r[:, mi, :], ps[:], AF.Identity)

            def qproj(j):  # bias via ACT (per-partition bias)
                sl = slice(j * NCH, (j + 1) * NCH)
                for ct in range(CT):
                    ps = pp2.tile([128, NCH], F32, tag="st", name=f"pq{j}_{ct}")
                    for ci in range(CT):
                        nc.tensor.matmul(ps[:], wsl(WQK, ci, slice(ct * 128, ct * 128 + 128)),
                                         xq_r[:, ci, sl], start=(ci == 0), stop=(ci == CT - 1))
                    nc.scalar.activation(q_r[:, ct, sl], ps[:], AF.Identity,
                                         bias=bias_s[:, ct, BQ:BQ + 1])

            av_tiles = {}
            acc_tiles = {}
            smp_tiles = {}
            recip_tiles = {}

            att_pend = []

            def att_sp(j, mi):
                # scores + exp; the av/acc consumption is deferred (skew) so
                # the av matmuls never wait on the ACT exp latency.
                sl = slice(j * NCH, (j + 1) * NCH)
                sp = pp2.tile([128, NCH], F32, tag="st", name=f"sp{j}_{mi}")
                for ci in range(CT):
                    nc.tensor.matmul(sp[:], xkv_r[:, ci, mi * 128:(mi + 1) * 128],
                                     q_r[:, ci, sl], start=(ci == 0), stop=(ci == CT - 1))
                et = epool.tile([128, NCH], CDT, tag="et", name=f"et{j}_{mi}")
                nc.scalar.activation(et[:], sp[:], AF.Exp)
                att_pend.append((j, mi, et))

            def att_av():
                j, mi, et = att_pend.pop(0)
                if mi == 0:
                    av_tiles[j] = (
                        pp.tile([128, NCH], F32, tag="av0", name=f"av0_{j}"),
                        pp.tile([128, NCH], F32, tag="av1", name=f"av1_{j}"),
                    )
                    acc_tiles[j] = apool.tile([128, NCH], CDT, tag="acc",
                                              name=f"acc{j}")
                av0, av1 = av_tiles[j]
                first, last = mi == 0, mi == MT - 1
                nc.tensor.matmul(av0[:], vt_r[:, mi, 0:128], et[:], start=first, stop=last)
                nc.tensor.matmul(av1[:], vt_r[:, mi, 128:256], et[:], start=first, stop=last)
                acc = acc_tiles[j]
                if first:
                    nc.vector.tensor_copy(acc[:], et[:])
                else:
                    nc.vector.tensor_add(acc[:], acc[:], et[:])

            def denom(j):  # one 1-row matmul reduces acc over the key axis
                smp = pp2.tile([1, NCH], F32, tag="ffn", name=f"smp{j}", bufs=1)
                nc.tensor.matmul(smp[:], ones_r[:], acc_tiles.pop(j)[:],
                                 start=True, stop=True)
                smp_tiles[j] = smp

            # ---- FFN for chunks 0..NJ-2, emitted during the next chunk ----
            def ffn_stages(j):
                st_ = {}
                sl = slice(j * NCH, (j + 1) * NCH)

                def s_recip():
                    smp = smp_tiles.pop(j)
                    rt = wpool.tile([1, NCH], F32, tag="recip_f", name=f"recipf{j}")
                    with nc.allow_low_precision(reason="softmax denom needs ~8 bits"):
                        nc.vector.reciprocal_approx_fast(rt[:], smp[:])
                    r = wpool.tile([1, NCH], CDT, tag="recip", name=f"recip{j}")
                    nc.vector.tensor_copy(r[:], rt[:])
                    recip_tiles[j] = r

                def s_rbp():
                    rbp = pp2.tile([128, NCH], F32, tag="ffn", name=f"rbp{j}", bufs=1)
                    nc.tensor.matmul(rbp[:], onesrow[:], recip_tiles.pop(j)[:],
                                     start=True, stop=True)
                    st_["rbp"] = rbp

                def s_avn():
                    rb = wpool.tile([128, NCH], F32, tag="rb", name=f"rb{j}")
                    nc.vector.tensor_copy(rb[:], st_["rbp"][:])
                    avn = wpool.tile([128, CT, NCH], CDT, tag="avn", name=f"avn{j}")
                    av0, av1 = av_tiles.pop(j)
                    nc.vector.tensor_mul(avn[:, 0, :], av0[:], rb[:])
                    nc.vector.tensor_mul(avn[:, 1, :], av1[:], rb[:])
                    st_["avn"] = avn
                    st_["hid"] = wpool.tile([128, CT, NCH], CDT, tag="hid",
                                            name=f"hid{j}")
                    st_["outp"] = wpool.tile([128, CT, NCH], F32, tag="outp",
                                             name=f"outp{j}")

                def s_hid(ot):
                    def go():
                        hp = pp2.tile([128, NCH], F32, tag="ffn", name=f"hp{j}_{ot}", bufs=1)
                        for ci in range(CT):
                            nc.tensor.matmul(
                                hp[:], wsl(W1, ci, slice(ot * 128, ot * 128 + 128)),
                                st_["avn"][:, ci, :], start=(ci == 0), stop=(ci == CT - 1))
                        nc.vector.tensor_scalar(st_["hid"][:, ot, :], hp[:],
                                                bias_s[:, ot, B1:B1 + 1], 0.0,
                                                mybir.AluOpType.add,
                                                mybir.AluOpType.max)
                    return go

                def s_out(ot):
                    def go():
                        op = pp2.tile([128, NCH], F32, tag="ffn", name=f"op{j}_{ot}", bufs=1)
                        for ci in range(CT):
                            nc.tensor.matmul(
                                op[:], wsl(W2, ci, slice(ot * 128, ot * 128 + 128)),
                                st_["hid"][:, ci, :], start=(ci == 0), stop=(ci == CT - 1))
                        nc.vector.tensor_scalar_add(st_["outp"][:, ot, :], op[:],
                                                    bias_s[:, ot, B2:B2 + 1])
                    return go

                def s_dma():
                    nc.sync.dma_start(out_d[:, :, sl], st_["outp"][:])

                return [(0, s_recip), (2, s_rbp), (4, s_avn),
                        (7, s_hid(0)), (9, s_hid(1)),
                        (11, s_out(0)), (13, s_out(1)), (15, s_dma)]

            # ---- tail FFN for the last chunk: normalization folded into the
            # rank-1 bias*denom matmuls so the reciprocal chain overlaps ----
            def tail_ffn(j):
                sl = slice(j * NCH, (j + 1) * NCH)
                av0, av1 = av_tiles.pop(j)
                avs = wpool.tile([128, CT, NCH], CDT, tag="avn", name="avs_t")
                nc.vector.tensor_copy(avs[:, 0, :], av0[:])
                nc.vector.tensor_copy(avs[:, 1, :], av1[:])
                smp = smp_tiles.pop(j)
                dvec = wpool.tile([1, NCH], CDT, tag="recip", name="dvec_t")
                nc.vector.tensor_copy(dvec[:], smp[:])
                rt = wpool.tile([1, NCH], F32, tag="recip_f", name="recipf_t")
                with nc.allow_low_precision(reason="softmax denom needs ~8 bits"):
                    nc.vector.reciprocal_approx_fast(rt[:], smp[:])
                rbp = pp2.tile([128, NCH], F32, tag="ffn", name="rbp_t", bufs=1)
                nc.tensor.matmul(rbp[:], onesrow_f[:], rt[:], start=True, stop=True)
                rb = wpool.tile([128, NCH], F32, tag="rb", name="rb_t")
                nc.vector.tensor_copy(rb[:], rbp[:])

                def warm(wi):
                    wps = pp.tile([128, NCH], F32, tag="av0", name=f"tw{wi}")
                    nc.tensor.matmul(wps[:], junk[:, 0:128], junk[:],
                                     start=True, stop=True)

                hid = wpool.tile([128, CT, NCH], CDT, tag="hid", name="hid_t")
                for ot in range(CT):
                    warm(2 * ot)
                    warm(2 * ot + 1)
                    hp = pp2.tile([128, NCH], F32, tag="st", name=f"hp_t{ot}")
                    for ci in range(CT):
                        nc.tensor.matmul(
                            hp[:], wsl(W1, ci, slice(ot * 128, ot * 128 + 128)),
                            avs[:, ci, :], start=(ci == 0), stop=False)
                    nc.tensor.matmul(hp[:], brow[:, ot * 128:ot * 128 + 128],
                                     dvec[:], start=False, stop=True)
                    nc.scalar.activation(hid[:, ot, :], hp[:], AF.Relu)
                outp = wpool.tile([128, CT, NCH], F32, tag="outp", name="outp_t")
                for ot in range(CT):
                    warm(4 + 4 * ot)
                    warm(5 + 4 * ot)
                    op = pp2.tile([128, NCH], F32, tag="st", name=f"op_t{ot}")
                    for ci in range(CT):
                        nc.tensor.matmul(
                            op[:], wsl(W2, ci, slice(ot * 128, ot * 128 + 128)),
                            hid[:, ci, :], start=(ci == 0), stop=False)
                    nc.tensor.matmul(op[:], brow[:, C + ot * 128:C + ot * 128 + 128],
                                     dvec[:], start=False, stop=True)
                    nc.vector.tensor_mul(outp[:, ot, :], op[:], rb[:])
                    nc.sync.dma_start(out_d[:, ot, sl], outp[:, ot, :])

            # ---- schedule ----
            # streaming prologue at 512-col granularity; chunk-0 attention
            # runs right behind the projections of each arriving kv slab.
            SKEW = 3
            for c in range(8):
                if c < NJ:
                    qproj(c)
                for mi in range(4 * c, 4 * c + 4):
                    vtproj(mi)
                for mi in range(4 * c, 4 * c + 4):
                    att_sp(0, mi)
                    if len(att_pend) > SKEW:
                        att_av()
            while att_pend:
                att_av()
            denom(0)

            for j in range(1, NJ):
                pending = ffn_stages(j - 1)
                for mi in range(MT):
                    att_sp(j, mi)
                    if len(att_pend) > SKEW:
                        att_av()
                    while pending and pending[0][0] == mi:
                        pending.pop(0)[1]()
                while att_pend:
                    att_av()
                denom(j)

            tail_ffn(NJ - 1)
    nc.compile()
    return nc


_NC_CACHE = None


def _get_nc():
    global _NC_CACHE
    if _NC_CACHE is None:
        _NC_CACHE = _build()
    return _NC_CACHE


def _fold(a):
    """[C, X] -> [128, CT, X] with channel tile as middle dim, contiguous."""
    x = np.ascontiguousarray(np.asarray(a, dtype=np.float32))
    return np.ascontiguousarray(x.reshape(CT, 128, -1).transpose(1, 0, 2))


def _make_in_maps(inputs):
    query_input = np.asarray(inputs["query_input"], np.float32).reshape(B, C, N)
    key_value_input = np.asarray(inputs["key_value_input"], np.float32).reshape(B, C, N)
    Wk64 = np.asarray(inputs["Wk"], np.float64)
    Wq64 = np.asarray(inputs["Wq"], np.float64)
    Wqk = (Wk64.T @ Wq64).astype(np.float32)  # scores = xkv^T (Wqk xq + Wk^T bq)
    wpack = _fold(np.concatenate(
        [Wqk.T] + [np.asarray(inputs[w], np.float32).T for w in ("Wv", "W1", "W2")],
        axis=1))  # [128, CT, 4C] in WQK,WV,W1,W2 order
    W1_ = np.asarray(inputs["W1"], np.float32)
    b1p = np.asarray(inputs["b1"], np.float32) + W1_ @ np.asarray(inputs["bv"], np.float32)
    b2_ = np.asarray(inputs["b2"], np.float32)
    bq2 = (Wk64.T @ np.asarray(inputs["bq"], np.float64)).astype(np.float32)
    biasp = _fold(np.stack([bq2, b1p, b2_], axis=1))  # [128, CT, 3]
    biasrow = np.concatenate([b1p, b2_])[None, :]  # [1, 2C]
    base = {"wpack": wpack, "biasp": biasp,
            "biasrow": np.ascontiguousarray(biasrow, dtype=np.float32)}
    in_maps = []
    for core in range(8):
        b, h = divmod(core, 2)
        m = dict(base)
        m["xq"] = _fold(query_input[b][:, h * NL:(h + 1) * NL])
        m["xkv"] = _fold(key_value_input[b])
        in_maps.append(m)
    return in_maps


def kernel(query_input, key_value_input, Wq, bq, Wk, bk, Wv, bv, W1, b1, W2, b2):
    in_maps = _make_in_maps(dict(
        query_input=query_input, key_value_input=key_value_input,
        Wq=Wq, bq=bq, Wk=Wk, bk=bk, Wv=Wv, bv=bv, W1=W1, b1=b1, W2=W2, b2=b2))
    nc = _get_nc()
    res = run_bass_kernel_spmd(nc, in_maps, core_ids=list(range(8)))

    out = np.empty((B, C, N), dtype=np.float32)
    for core in range(8):
        b, h = divmod(core, 2)
        o = res.results[core]["out"]  # [128, CT, NL]
        out[b][:, h * NL:(h + 1) * NL] = o.transpose(1, 0, 2).reshape(C, NL)
    return out.reshape(B, C, N).reshape(B, C, H, W)


# revision 22
# speedup vs baseline: 1.1883x; 1.1883x over previous
"""Fused conv1x1-attention-FFN kernel for 8 trn2 NeuronCores.

Reference computation (per batch b of 4, N = 64*64 = 4096 pixels, C = 256):
    q = Wq @ x_q + bq ; k = Wk @ x_kv + bk ; v = Wv @ x_kv + bv      [C, N]
    attn = softmax_over_keys(q^T k)                                   [N, N]
    av = v @ attn^T                                                   [C, N]
    out = W2 @ relu(W1 @ av + b1) + b2                                [C, N]

Sharding: 8 cores = 4 batches x 2 query-row halves. Each core computes the
full K/V for its batch (cheap vs the [N,N] attention) and attends its 2048
query rows against all 4096 keys — no collectives.

Bias algebra (host-side): bk shifts every score of a query equally, which
softmax cancels -> dropped. bv adds bv per channel to the normalized
attention output (weights sum to 1) -> folded into b1' = b1 + W1 @ bv.

On-chip layout: scores are computed TRANSPOSED, S^T[m,n] = sum_c k[c,m]
q[c,n], so av[c,n] = sum_m v^T[m,c] E[m,n] needs no transpose; v is
projected into v^T[m,c] by using x_kv tiles as the stationary operand.
Softmax denominators are accumulated on the Vector engine (acc += exp
tile) and reduced over keys with ONE 1-column matmul per chunk.

Weight algebra (host-side): the k-projection is eliminated entirely —
scores are bilinear, so S^T = x_kv^T (Wqk x_q + Wk^T bq) with Wqk =
Wk^T Wq precomputed on host; the score matmuls use raw x_kv tiles as the
stationary operand. Weights are packed (Wqk,Wv,W1,W2) and DMA'd in
slices interleaved with the input chunks so the q-projection starts as
soon as Wqk plus 512 columns of x_q have landed; chunk-0 attention
streams right behind the projections, with the av/acc consumption skewed
SKEW key-tiles behind the score/exp pipeline so the av matmuls never
wait on ACT latency. Junk warm-up matmuls run during the initial DMA
window (and through the latency-bound tail) to keep the PE HAM clock
gate released at 2.4 GHz. Tail: the last chunk's FFN runs UN-normalized — relu(s*z
+ b1) = s*relu(z + b1*denom) for s>0 — with b1*denom / b2*denom added via
rank-1 matmuls into PSUM, so the reciprocal/broadcast chain overlaps the
FFN matmuls instead of preceding them; a final elementwise multiply by
the broadcast reciprocal lands the result.

Everything DMAs straight into float32r tiles (bit-identical to f32) — no
staging casts. float32r matmuls run 1 PE cycle/row at free-dim >= 256;
PSUM accumulates fp32.
"""
import sys

sys.path.insert(0, "/opt/trn_rl_repo")

import numpy as np
from concourse import bass, bacc, mybir, tile
from concourse.bass_utils import run_bass_kernel_spmd

F32 = mybir.dt.float32
CDT = mybir.dt.float32r

B, C, H, W = 4, 256, 64, 64
N = H * W              # 4096 keys per batch
NL = N // 2            # 2048 query rows per core
CT = C // 128          # 2 channel tiles
MT = N // 128          # 32 key tiles
NCH = 512              # query-column chunk
NJ = NL // NCH         # 4 chunks
AF = mybir.ActivationFunctionType
BQ, B1, B2 = range(3)  # bias pack columns
WQK, WV, W1, W2 = range(4)  # weight pack order (DMA priority order)


def _build():
    nc = bacc.Bacc(None, target_bir_lowering=False, debug=False)

    xq_d = nc.declare_dram_parameter("xq", [128, CT, NL], CDT, isOutput=False)
    xkv_d = nc.declare_dram_parameter("xkv", [128, CT, N], CDT, isOutput=False)
    wp_d = nc.declare_dram_parameter("wpack", [128, CT, 4 * C], CDT, isOutput=False)
    bias_d = nc.declare_dram_parameter("biasp", [128, CT, 3], F32, isOutput=False)
    brow_d = nc.declare_dram_parameter("biasrow", [1, 2 * C], CDT, isOutput=False)
    out_d = nc.declare_dram_parameter("out", [128, CT, NL], F32, isOutput=True)

    with tile.TileContext(nc) as tc:
        with (
            tc.tile_pool(name="const", bufs=1) as cpool,
            tc.tile_pool(name="big", bufs=1) as bpool,
            tc.tile_pool(name="work", bufs=2) as wpool,
            tc.tile_pool(name="et", bufs=6) as epool,
            tc.tile_pool(name="acc", bufs=2) as apool,
            tc.tile_pool(name="psA", bufs=2, space="PSUM") as pp,
            tc.tile_pool(name="psB", bufs=3, space="PSUM") as pp2,
        ):
            w_r = cpool.tile([128, CT, 4 * C], CDT, tag="w_r")
            bias_s = cpool.tile([128, CT, 3], F32, tag="bias_s")
            brow = cpool.tile([1, 2 * C], CDT, tag="brow")
            xkv_r = bpool.tile([128, CT, N], CDT, tag="xkv_r")
            xq_r = bpool.tile([128, CT, NL], CDT, tag="xq_r")

            def wslice(i0, i1):
                return (w_r[:, :, i0 * C:i1 * C], wp_d[:, :, i0 * C:i1 * C])

            # DMA issue order == arrival order; interleave so the first
            # projections can start after ~1.3MB instead of ~2.6MB.
            nc.sync.dma_start(*wslice(WQK, WQK + 1))
            nc.sync.dma_start(xq_r[:, :, 0:512], xq_d[:, :, 0:512])
            nc.sync.dma_start(*wslice(WV, WV + 1))
            nc.sync.dma_start(xkv_r[:, :, 0:512], xkv_d[:, :, 0:512])
            nc.sync.dma_start(bias_s[:], bias_d[:])
            nc.sync.dma_start(xkv_r[:, :, 512:1024], xkv_d[:, :, 512:1024])
            nc.sync.dma_start(*wslice(W1, W2 + 1))
            nc.sync.dma_start(xq_r[:, :, 512:1024], xq_d[:, :, 512:1024])
            for c in range(1, 4):
                nc.sync.dma_start(xkv_r[:, :, c * 1024:(c + 1) * 1024],
                                  xkv_d[:, :, c * 1024:(c + 1) * 1024])
                if c == 1:
                    nc.sync.dma_start(xq_r[:, :, 1024:2048],
                                      xq_d[:, :, 1024:2048])
            nc.sync.dma_start(brow[:], brow_d[:])  # tail-only bias rows

            def wsl(idx, ct, osl):
                return w_r[:, ct, idx * C + osl.start: idx * C + osl.stop]

            # PE warm-up: the HAM clock gate starts at 1.2 GHz and needs
            # ~3.4us of sustained activity to release to 2.4 GHz. The PE is
            # otherwise idle 8-14us waiting for the first DMAs, so burn that
            # window on junk matmuls and start the real stream at full clock.
            junk_f = cpool.tile([128, NCH], F32, tag="junk_f")
            nc.vector.memset(junk_f[:], 1.0)
            junk = cpool.tile([128, NCH], CDT, tag="junk")
            nc.vector.tensor_copy(junk[:], junk_f[:])
            for wi in range(12):
                wp_ps = pp2.tile([128, NCH], F32, tag="st", name=f"warm{wi}")
                nc.tensor.matmul(wp_ps[:], junk[:, 0:128], junk[:],
                                 start=True, stop=True)

            ones_f = cpool.tile([128, 1], F32, tag="ones_f")
            nc.vector.memset(ones_f[:], 1.0)
            ones_r = cpool.tile([128, 1], CDT, tag="ones_r")
            nc.vector.tensor_copy(ones_r[:], ones_f[:])
            onesrow_f = cpool.tile([1, 128], F32, tag="onesrow_f")
            nc.vector.memset(onesrow_f[:], 1.0)
            onesrow = cpool.tile([1, 128], CDT, tag="onesrow")
            nc.vector.tensor_copy(onesrow[:], onesrow_f[:])

            q_r = bpool.tile([128, CT, NL], CDT, tag="q_r")
            vt_r = bpool.tile([128, MT, C], CDT, tag="vt_r")

            def vtproj(mi):  # v^T tile: [key, channel] layout, no bias
                ps = pp2.tile([128, C], F32, tag="st", name=f"pv{mi}")
                for ci in range(CT):
                    nc.tensor.matmul(ps[:], xkv_r[:, ci, mi * 128:(mi + 1) * 128],
                                     w_r[:, ci, WV * C:WV * C + C], start=(ci == 0), stop=(ci == CT - 1))
                nc.scalar.activation(vt_r[:, mi, :], ps[:], AF.Identity)

            def qproj(j):  # bias via ACT (per-partition bias)
                sl = slice(j * NCH, (j + 1) * NCH)
                for ct in range(CT):
                    ps = pp2.tile([128, NCH], F32, tag="st", name=f"pq{j}_{ct}")
                    for ci in range(CT):
                        nc.tensor.matmul(ps[:], wsl(WQK, ci, slice(ct * 128, ct * 128 + 128)),
                                         xq_r[:, ci, sl], start=(ci == 0), stop=(ci == CT - 1))
                    nc.scalar.activation(q_r[:, ct, sl], ps[:], AF.Identity,
                                         bias=bias_s[:, ct, BQ:BQ + 1])

            av_tiles = {}
            acc_tiles = {}
            smp_tiles = {}
            recip_tiles = {}

            att_pend = []

            def att_sp(j, mi):
                # scores + exp; the av/acc consumption is deferred (skew) so
                # the av matmuls never wait on the ACT exp latency.
                sl = slice(j * NCH, (j + 1) * NCH)
                sp = pp2.tile([128, NCH], F32, tag="st", name=f"sp{j}_{mi}")
                for ci in range(CT):
                    nc.tensor.matmul(sp[:], xkv_r[:, ci, mi * 128:(mi + 1) * 128],
                                     q_r[:, ci, sl], start=(ci == 0), stop=(ci == CT - 1))
                et = epool.tile([128, NCH], CDT, tag="et", name=f"et{j}_{mi}")
                nc.scalar.activation(et[:], sp[:], AF.Exp)
                att_pend.append((j, mi, et))

            def att_av():
                j, mi, et = att_pend.pop(0)
                if mi == 0:
                    av_tiles[j] = (
                        pp.tile([128, NCH], F32, tag="av0", name=f"av0_{j}"),
                        pp.tile([128, NCH], F32, tag="av1", name=f"av1_{j}"),
                    )
                    acc_tiles[j] = apool.tile([128, NCH], CDT, tag="acc",
                                              name=f"acc{j}")
                av0, av1 = av_tiles[j]
                first, last = mi == 0, mi == MT - 1
                nc.tensor.matmul(av0[:], vt_r[:, mi, 0:128], et[:], start=first, stop=last)
                nc.tensor.matmul(av1[:], vt_r[:, mi, 128:256], et[:], start=first, stop=last)
                acc = acc_tiles[j]
                if first:
                    nc.vector.tensor_copy(acc[:], et[:])
                else:
                    nc.vector.tensor_add(acc[:], acc[:], et[:])

            def denom(j):  # one 1-row matmul reduces acc over the key axis
                smp = pp2.tile([1, NCH], F32, tag="ffn", name=f"smp{j}", bufs=1)
                nc.tensor.matmul(smp[:], ones_r[:], acc_tiles.pop(j)[:],
                                 start=True, stop=True)
                smp_tiles[j] = smp

            # ---- FFN for chunks 0..NJ-2, emitted during the next chunk ----
            def ffn_stages(j):
                st_ = {}
                sl = slice(j * NCH, (j + 1) * NCH)

                def s_recip():
                    smp = smp_tiles.pop(j)
                    rt = wpool.tile([1, NCH], F32, tag="recip_f", name=f"recipf{j}")
                    with nc.allow_low_precision(reason="softmax denom needs ~8 bits"):
                        nc.vector.reciprocal_approx_fast(rt[:], smp[:])
                    r = wpool.tile([1, NCH], CDT, tag="recip", name=f"recip{j}")
                    nc.vector.tensor_copy(r[:], rt[:])
                    recip_tiles[j] = r

                def s_rbp():
                    rbp = pp2.tile([128, NCH], F32, tag="ffn", name=f"rbp{j}", bufs=1)
                    nc.tensor.matmul(rbp[:], onesrow[:], recip_tiles.pop(j)[:],
                                     start=True, stop=True)
                    st_["rbp"] = rbp

                def s_avn():
                    rb = wpool.tile([128, NCH], F32, tag="rb", name=f"rb{j}")
                    nc.vector.tensor_copy(rb[:], st_["rbp"][:])
                    avn = wpool.tile([128, CT, NCH], CDT, tag="avn", name=f"avn{j}")
                    av0, av1 = av_tiles.pop(j)
                    nc.vector.tensor_mul(avn[:, 0, :], av0[:], rb[:])
                    nc.vector.tensor_mul(avn[:, 1, :], av1[:], rb[:])
                    st_["avn"] = avn
                    st_["hid"] = wpool.tile([128, CT, NCH], CDT, tag="hid",
                                            name=f"hid{j}")
                    st_["outp"] = wpool.tile([128, CT, NCH], F32, tag="outp",
                                             name=f"outp{j}")

                def s_hid(ot):
                    def go():
                        hp = pp2.tile([128, NCH], F32, tag="ffn", name=f"hp{j}_{ot}", bufs=1)
                        for ci in range(CT):
                            nc.tensor.matmul(
                                hp[:], wsl(W1, ci, slice(ot * 128, ot * 128 + 128)),
                                st_["avn"][:, ci, :], start=(ci == 0), stop=(ci == CT - 1))
                        nc.vector.tensor_scalar(st_["hid"][:, ot, :], hp[:],
                                                bias_s[:, ot, B1:B1 + 1], 0.0,
                                                mybir.AluOpType.add,
                                                mybir.AluOpType.max)
                    return go

                def s_out(ot):
                    def go():
                        op = pp2.tile([128, NCH], F32, tag="ffn", name=f"op{j}_{ot}", bufs=1)
                        for ci in range(CT):
                            nc.tensor.matmul(
                                op[:], wsl(W2, ci, slice(ot * 128, ot * 128 + 128)),
                                st_["hid"][:, ci, :], start=(ci == 0), stop=(ci == CT - 1))
                        nc.vector.tensor_scalar_add(st_["outp"][:, ot, :], op[:],
                                                    bias_s[:, ot, B2:B2 + 1])
                    return go

                def s_dma():
                    nc.sync.dma_start(out_d[:, :, sl], st_["outp"][:])

                return [(0, s_recip), (2, s_rbp), (4, s_avn),
                        (7, s_hid(0)), (9, s_hid(1)),
                        (11, s_out(0)), (13, s_out(1)), (15, s_dma)]

            # ---- tail FFN for the last chunk: normalization folded into the
            # rank-1 bias*denom matmuls so the reciprocal chain overlaps ----
            def tail_ffn(j):
                sl = slice(j * NCH, (j + 1) * NCH)
                av0, av1 = av_tiles.pop(j)
                avs = wpool.tile([128, CT, NCH], CDT, tag="avn", name="avs_t")
                nc.vector.tensor_copy(avs[:, 0, :], av0[:])
                nc.vector.tensor_copy(avs[:, 1, :], av1[:])
                smp = smp_tiles.pop(j)
                dvec = wpool.tile([1, NCH], CDT, tag="recip", name="dvec_t")
                nc.vector.tensor_copy(dvec[:], smp[:])
                rt = wpool.tile([1, NCH], F32, tag="recip_f", name="recipf_t")
                with nc.allow_low_precision(reason="softmax denom needs ~8 bits"):
                    nc.vector.reciprocal_approx_fast(rt[:], smp[:])
                rbp = pp2.tile([128, NCH], F32, tag="ffn", name="rbp_t", bufs=1)
                nc.tensor.matmul(rbp[:], onesrow_f[:], rt[:], start=True, stop=True)
                rb = wpool.tile([128, NCH], F32, tag="rb", name="rb_t")
                nc.vector.tensor_copy(rb[:], rbp[:])

                def warm(wi):
                    wps = pp.tile([128, NCH], F32, tag="av0", name=f"tw{wi}")
                    nc.tensor.matmul(wps[:], junk[:, 0:128], junk[:],
                                     start=True, stop=True)

                hid = wpool.tile([128, CT, NCH], CDT, tag="hid", name="hid_t")
                for ot in range(CT):
                    warm(2 * ot)
                    warm(2 * ot + 1)
                    hp = pp2.tile([128, NCH], F32, tag="st", name=f"hp_t{ot}")
                    for ci in range(CT):
                        nc.tensor.matmul(
                            hp[:], wsl(W1, ci, slice(ot * 128, ot * 128 + 128)),
                            avs[:, ci, :], start=(ci == 0), stop=False)
                    nc.tensor.matmul(hp[:], brow[:, ot * 128:ot * 128 + 128],
                                     dvec[:], start=False, stop=True)
                    nc.scalar.activation(hid[:, ot, :], hp[:], AF.Relu)
                outp = wpool.tile([128, CT, NCH], F32, tag="outp", name="outp_t")
                for ot in range(CT):
                    warm(4 + 4 * ot)
                    warm(5 + 4 * ot)
                    op = pp2.tile([128, NCH], F32, tag="st", name=f"op_t{ot}")
                    for ci in range(CT):
                        nc.tensor.matmul(
                            op[:], wsl(W2, ci, slice(ot * 128, ot * 128 + 128)),
                            hid[:, ci, :], start=(ci == 0), stop=False)
                    nc.tensor.matmul(op[:], brow[:, C + ot * 128:C + ot * 128 + 128],
                                     dvec[:], start=False, stop=True)
                    nc.vector.tensor_mul(outp[:, ot, :], op[:], rb[:])
                    nc.sync.dma_start(out_d[:, ot, sl], outp[:, ot, :])

            # ---- schedule ----
            # streaming prologue at 512-col granularity; chunk-0 attention
            # runs right behind the projections of each arriving kv slab.
            SKEW = 4
            for c in range(8):
                if c < NJ:
                    qproj(c)
                for mi in range(4 * c, 4 * c + 4):
                    vtproj(mi)
                for mi in range(4 * c, 4 * c + 4):
                    att_sp(0, mi)
                    if len(att_pend) > SKEW:
                        att_av()
            while att_pend:
                att_av()
            denom(0)

            for j in range(1, NJ):
                pending = ffn_stages(j - 1)
                for mi in range(MT):
                    att_sp(j, mi)
                    if len(att_pend) > SKEW:
                        att_av()
                    while pending and pending[0][0] == mi:
                        pending.pop(0)[1]()
                while att_pend:
                    att_av()
                denom(j)

            tail_ffn(NJ - 1)
    nc.compile()
    return nc


_NC_CACHE = None


def _get_nc():
    global _NC_CACHE
    if _NC_CACHE is None:
        _NC_CACHE = _build()
    return _NC_CACHE


def _fold(a):
    """[C, X] -> [128, CT, X] with channel tile as middle dim, contiguous."""
    x = np.ascontiguousarray(np.asarray(a, dtype=np.float32))
    return np.ascontiguousarray(x.reshape(CT, 128, -1).transpose(1, 0, 2))


def _make_in_maps(inputs):
    query_input = np.asarray(inputs["query_input"], np.float32).reshape(B, C, N)
    key_value_input = np.asarray(inputs["key_value_input"], np.float32).reshape(B, C, N)
    Wk64 = np.asarray(inputs["Wk"], np.float64)
    Wq64 = np.asarray(inputs["Wq"], np.float64)
    Wqk = (Wk64.T @ Wq64).astype(np.float32)  # scores = xkv^T (Wqk xq + Wk^T bq)
    wpack = _fold(np.concatenate(
        [Wqk.T] + [np.asarray(inputs[w], np.float32).T for w in ("Wv", "W1", "W2")],
        axis=1))  # [128, CT, 4C] in WQK,WV,W1,W2 order
    W1_ = np.asarray(inputs["W1"], np.float32)
    b1p = np.asarray(inputs["b1"], np.float32) + W1_ @ np.asarray(inputs["bv"], np.float32)
    b2_ = np.asarray(inputs["b2"], np.float32)
    bq2 = (Wk64.T @ np.asarray(inputs["bq"], np.float64)).astype(np.float32)
    biasp = _fold(np.stack([bq2, b1p, b2_], axis=1))  # [128, CT, 3]
    biasrow = np.concatenate([b1p, b2_])[None, :]  # [1, 2C]
    base = {"wpack": wpack, "biasp": biasp,
            "biasrow": np.ascontiguousarray(biasrow, dtype=np.float32)}
    in_maps = []
    for core in range(8):
        b, h = divmod(core, 2)
        m = dict(base)
        m["xq"] = _fold(query_input[b][:, h * NL:(h + 1) * NL])
        m["xkv"] = _fold(key_value_input[b])
        in_maps.append(m)
    return in_maps


def kernel(query_input, key_value_input, Wq, bq, Wk, bk, Wv, bv, W1, b1, W2, b2):
    in_maps = _make_in_maps(dict(
        query_input=query_input, key_value_input=key_value_input,
        Wq=Wq, bq=bq, Wk=Wk, bk=bk, Wv=Wv, bv=bv, W1=W1, b1=b1, W2=W2, b2=b2))
    nc = _get_nc()
    res = run_bass_kernel_spmd(nc, in_maps, core_ids=list(range(8)))

    out = np.empty((B, C, N), dtype=np.float32)
    for core in range(8):
        b, h = divmod(core, 2)
        o = res.results[core]["out"]  # [128, CT, NL]
        out[b][:, h * NL:(h + 1) * NL] = o.transpose(1, 0, 2).reshape(C, NL)
    return out.reshape(B, C, N).reshape(B, C, H, W)
